# revision 1
# baseline (speedup 1.0000x reference)
"""Trainium2 Bass kernel for a dense transformer block (LN->QKV->causal attn->proj
-> residual -> LN -> MLP(gelu tanh) -> residual).

Sharding: 8 independent cores = 4 batches x 2 query-halves. No collectives.
Each core processes its 512 query rows against a locally reordered KV sequence
(diagonal 512 rows first, prefix context after; zero padding + data mask for the
lower half), so every core runs an identical instruction stream and the causal
mask is a compile-time affine_select.

v2: QKV / attn-V / proj matmuls run in fp8e4m3 DoubleRow (K=256 per pass);
scores run plain fp8 (K=64 per head). Weights are scaled x32 host-side to avoid
fp8 denormals; the scale folds into the exp() argument and the softmax
denominator ones-column (=32). The V operand is padded to 128 columns
(64 dims + ones + 63 zeros) because DoubleRow needs out-partitions in
{32,64,128}. MLP stays bf16 (fp8 would exceed the error budget). Causal
pair-trimming skips the never-visible quarter of the diagonal work with AV
accumulation order [pair2,pair3,pair1,pair0] keeping PSUM start/stop exact.
LN normalization runs on ACT (scale+bias per partition); DMA issues are
batched (10 weight blocks per descriptor) to keep the SP sequencer off the
critical path.
"""

import math
import sys
from dataclasses import dataclass

import numpy as np

sys.path.insert(0, "/opt/trn_rl_repo")

import concourse.bacc as bacc  # noqa: E402
import concourse.bass as bass  # noqa: E402
import concourse.tile as tile  # noqa: E402
from concourse import mybir  # noqa: E402

F32 = mybir.dt.float32
BF16 = mybir.dt.bfloat16
F32R = mybir.dt.float32r
F8 = mybir.dt.float8e4
AF = mybir.ActivationFunctionType
ALU = mybir.AluOpType
DR = mybir.MatmulPerfMode.DoubleRow

EPS = 1e-5
DH = 64   # head dim (fixed)
SW = 32.0  # fp8 weight scale
USE_DR = True  # fp8 DoubleRow vs plain fp8 matmul pairs (A/B testable)


@dataclass(frozen=True)
class Cfg:
    Sq: int = 512     # query rows per core
    Skv: int = 1024   # local KV rows per core
    D: int = 1280     # model dim
    H: int = 20       # heads
    Dff: int = 5120   # MLP hidden

    @property
    def ND(self):
        return self.D // 128      # D chunks (10)

    @property
    def NP(self):
        return self.ND // 2       # DR k-pairs for D contraction (5)

    @property
    def NKB(self):
        return self.Skv // 128    # kv blocks (8)

    @property
    def NDIAG(self):
        return self.Sq // 128     # diagonal kv blocks (4)

    @property
    def NQB(self):
        return self.Sq // 128     # query row blocks (4)

    @property
    def NG5(self):
        return self.H // 4        # head groups of 4 (5)

    @property
    def NHP(self):
        return self.H // 2        # head pairs (10)

    @property
    def NCC(self):
        return 12                 # proj contraction chunks (10 data + 2 pad)

    @property
    def NHT(self):
        return self.Dff // 128    # MLP hidden blocks (40)

    @property
    def NG(self):
        return self.D // math.gcd(512, self.D)  # bn_stats subgroups


def tiles_of(total, maxw=512):
    out = []
    c = 0
    while c < total:
        w = min(maxw, total - c)
        out.append((c, w))
        c += w
    return out


def build_program(cfg: Cfg, has_bqk: bool, has_bv: bool, has_bproj: bool,
                  has_bfc2: bool, repeat: int = 1):
    c = cfg
    nc = bacc.Bacc("TRN2", target_bir_lowering=False, debug=False, num_devices=8)

    hid_d = nc.dram_tensor("hid", [c.Skv, c.D], BF16, kind="ExternalInput").ap()
    kvm_d = nc.dram_tensor("kvmask", [c.NKB, 128], F32, kind="ExternalInput").ap()
    # per group g (4 heads): cols = [q heads 4g..4g+3 | k heads 4g..4g+3]
    wqk_d = nc.dram_tensor("wqk8", [c.NG5, c.D, 512], F8, kind="ExternalInput").ap()
    wv_d = nc.dram_tensor("wv8", [c.NG5, c.D, 256], F8, kind="ExternalInput").ap()
    wvb_d = nc.dram_tensor("wvb", [c.NG5, c.D, 256], BF16, kind="ExternalInput").ap()
    wproj_d = nc.dram_tensor("wproj8", [c.NCC * 128, c.D], F8, kind="ExternalInput").ap()
    wprojb_d = nc.dram_tensor("wprojb", [c.NCC * 128, c.D], BF16, kind="ExternalInput").ap()
    wfc_d = nc.dram_tensor("wfc", [c.D, c.Dff], BF16, kind="ExternalInput").ap()
    wfc2_d = nc.dram_tensor("wfc2", [c.Dff, c.D], BF16, kind="ExternalInput").ap()
    bfc_d = nc.dram_tensor("bfc", [c.NHT, 128], F32, kind="ExternalInput").ap()
    ident_d = nc.dram_tensor("ident", [128, 128], F32R, kind="ExternalInput").ap()
    out_d = nc.dram_tensor("out", [c.Sq, c.D], F32, kind="ExternalOutput").ap()
    bqk_d = bv_d = bfc2_d = None
    if has_bqk:
        # scaled x SW; columns in the same (g, q/k, block) order as wqk8
        bqk_d = nc.dram_tensor("bqk", [2 * c.ND, 128], F32, kind="ExternalInput").ap()
    if has_bv:
        bv_d = nc.dram_tensor("bv", [c.H, 64], F32, kind="ExternalInput").ap()
    if has_bfc2:
        bfc2_d = nc.dram_tensor("bfc2", [1, c.D], F32, kind="ExternalInput").ap()

    SEXP = 1.0 / (SW * SW * math.sqrt(DH))

    with tile.TileContext(nc) as tc, nc.allow_low_precision(
        reason="fp8/bf16 matmuls with fp32 PSUM accumulation"
    ):
        # -------- pools alive the whole kernel --------
        constp = tc.alloc_tile_pool(name="constp", bufs=1)
        workp = tc.alloc_tile_pool(name="workp", bufs=2)
        statp = tc.alloc_tile_pool(name="statp", bufs=2)
        wcache = tc.alloc_tile_pool(name="wcache", bufs=2)

        # constants
        ident_sb = constp.tile([128, 128], F32R, name="ident_sb")
        nc.sync.dma_start(out=ident_sb, in_=ident_d)
        kvm_sb = constp.tile([128, c.NKB], F32, name="kvm_sb")
        nc.sync.dma_start(out=kvm_sb, in_=kvm_d.rearrange("i p -> p i"))
        eps_sb = constp.tile([128, 1], F32, name="eps_sb")
        nc.vector.memset(eps_sb, EPS)
        warm_sb = constp.tile([128, 1], F32, name="warm_sb")
        nc.scalar.activation(out=warm_sb, in_=eps_sb, func=AF.Sqrt)
        ones_h = constp.tile([128, c.H, 1], F32, name="ones_h")
        nc.vector.memset(ones_h, SW)
        bfc_sb = constp.tile([128, c.NHT], F32, name="bfc_sb")
        nc.sync.dma_start(out=bfc_sb, in_=bfc_d.rearrange("i p -> p i"))
        bqk_sb = bv_sb = bfc2_sb = None
        if has_bqk:
            bqk_sb = constp.tile([128, 2 * c.ND], F32, name="bqk_sb")
            nc.sync.dma_start(out=bqk_sb, in_=bqk_d.rearrange("i p -> p i"))
        if has_bv:
            bv_sb = constp.tile([64, c.H], F32, name="bv_sb")
            nc.sync.dma_start(out=bv_sb, in_=bv_d.rearrange("h c -> c h"))
        if has_bfc2:
            bfc2_sb = constp.tile([128, c.D], F32, name="bfc2_sb")
            nc.gpsimd.dma_start(
                out=bfc2_sb,
                in_=bass.AP(
                    tensor=bfc2_d.tensor,
                    offset=bfc2_d.offset,
                    ap=[[0, 128], bfc2_d.ap[1]],
                ),
            )

        def layer_norm_tile(x_ap, xhat_ap, i):
            """row LN: xhat = (x - mean) * rsqrt(var + eps). Stats on DVE,
            the wide normalize on ACT (scale+bias per partition)."""
            stats = statp.tile([128, c.NG, 6], F32, name=f"stats_{i}", tag="stats")
            xg = x_ap.rearrange("p (g d) -> p g d", g=c.NG)
            for g in range(c.NG):
                nc.vector.bn_stats(out=stats[:, g, :], in_=xg[:, g, :])
            mv = statp.tile([128, 2], F32, name=f"mv_{i}", tag="mv")
            nc.vector.bn_aggr(out=mv, in_=stats)
            nc.scalar.activation(
                out=mv[:, 1:2], in_=mv[:, 1:2], func=AF.Sqrt, bias=eps_sb
            )
            nc.vector.reciprocal(out=mv[:, 1:2], in_=mv[:, 1:2])
            nmu = statp.tile([128, 1], F32, name=f"nmu_{i}", tag="nmu")
            nc.vector.tensor_scalar(
                out=nmu, in0=mv[:, 0:1], scalar1=mv[:, 1:2], scalar2=-1.0,
                op0=ALU.mult, op1=ALU.mult,
            )
            nc.scalar.activation(
                out=xhat_ap, in_=x_ap, func=AF.Identity,
                scale=mv[:, 1:2], bias=nmu,
            )

        def _emit_phases(rep):
            residp = tc.alloc_tile_pool(name=f"residp{rep}", bufs=1, side="right")
            resid32 = residp.tile([128, c.NQB, c.D], F32, name=f"resid32_{rep}")

            x8p = tc.alloc_tile_pool(name=f"x8p{rep}", bufs=1)
            x8T = x8p.tile([128, c.ND, c.Skv], F8, name="x8T")
            xTb = x8p.tile([128, c.ND, 256], BF16, name="xTb")
            wqkp = tc.alloc_tile_pool(name=f"wqkp{rep}", bufs=2)

            # ---- Phase A: LN1 + transpose -> x8T (fp8), resid32 = 32*hid ----
            ptp = tc.alloc_tile_pool(name=f"ptp{rep}", bufs=2, space="PSUM")
            for i in range(c.NKB):
                x_t = workp.tile([128, c.D], BF16, name="x_t", tag="row")
                nc.sync.dma_start(out=x_t, in_=hid_d[i * 128:(i + 1) * 128, :])
                if i < c.NQB:
                    nc.gpsimd.tensor_scalar_mul(
                        out=resid32[:, i, :], in0=x_t, scalar1=SW
                    )
                xhat = workp.tile([128, c.D], F32R, name="xhat", tag="row2")
                layer_norm_tile(x_t, xhat, i)
                for d0 in range(0, c.ND, 4):
                    nd = min(4, c.ND - d0)
                    pt = ptp.tile([128, 4, 128], F32R, name="pt", tag="pt")
                    for k in range(nd):
                        nc.tensor.transpose(
                            (pt[:, k, :]),
                            (xhat[:, (d0 + k) * 128:(d0 + k + 1) * 128]),
                            (ident_sb),
                        )
                    nc.scalar.activation(
                        out=x8T[:, d0:d0 + nd, i * 128:(i + 1) * 128],
                        in_=pt[:, 0:nd, :],
                        func=AF.Copy,
                    )
                    if i < 2:
                        nc.vector.tensor_copy(
                            out=xTb[:, d0:d0 + nd, i * 128:(i + 1) * 128],
                            in_=pt[:, 0:nd, :],
                        )
            ptp.release()

            # -------- Phase B pools --------
            ytp = tc.alloc_tile_pool(name=f"ytp{rep}", bufs=1, side="right")
            yT8 = ytp.tile([128, c.NCC, c.Sq], F8, name="yT8")
            yT8b = ytp.tile([128, c.NCC, 128], BF16, name="yT8b")
            wpbp = tc.alloc_tile_pool(name=f"wpbp{rep}", bufs=2, side="right")
            wpp = tc.alloc_tile_pool(name=f"wpp{rep}", bufs=2, side="right")
            qk8p = tc.alloc_tile_pool(name=f"qk8p{rep}", bufs=1)
            # chunk hp = head pair: head 2hp at partitions 0:64, 2hp+1 at 64:128
            QT8 = qk8p.tile([128, c.NHP, c.Sq], F8, name="QT8")
            KT8 = qk8p.tile([128, c.NHP, c.Skv], F8, name="KT8")
            V8 = qk8p.tile([128, c.NKB - 2, c.H, 128], F8, name="V8")
            Vb = qk8p.tile([128, 2, c.H, 128], BF16, name="Vb")
            e8p = tc.alloc_tile_pool(name=f"e8p{rep}", bufs=2, side="right")
            ynp = tc.alloc_tile_pool(name=f"ynp{rep}", bufs=2, side="right")
            drp = tc.alloc_tile_pool(name=f"drp{rep}", bufs=1, space="DRAM")
            dscr = drp.tile([c.H, c.Sq], F32R, name=f"dscr{rep}")

            pqk = tc.alloc_tile_pool(name=f"pqk{rep}", bufs=2, space="PSUM")
            pss = tc.alloc_tile_pool(name=f"pss{rep}", bufs=2, space="PSUM")
            pyv = tc.alloc_tile_pool(name=f"pyv{rep}", bufs=2, space="PSUM")

            # ones columns (=SW*kvm) and zero padding; yT8 pad chunks
            nc.vector.memset(V8[:, :, :, 65:128], 0.0)
            nc.vector.memset(Vb[:, :, :, 65:128], 0.0)
            for i in range(2):
                nc.vector.tensor_scalar_mul(
                    out=Vb[:, i, :, 64:65],
                    in0=ones_h,
                    scalar1=kvm_sb[:, i:i + 1],
                )
            for i in range(2, c.NKB):
                nc.vector.tensor_scalar_mul(
                    out=V8[:, i - 2, :, 64:65],
                    in0=ones_h,
                    scalar1=kvm_sb[:, i:i + 1],
                )
            nc.vector.memset(yT8[:, 10:12, :], 0.0)
            nc.vector.memset(yT8b[:, 10:12, :], 0.0)
            if has_bproj:
                # ones row: chunk 10 row 0 pairs with wproj8 row 1280 = bproj
                nc.vector.memset(yT8[0:1, 10, :], 1.0)
                nc.vector.memset(yT8b[0:1, 10, :], 1.0)

            fill0 = nc.gpsimd.to_reg(0.0)

            def emit_group_gemms(g):
                wqkg = wqkp.tile([128, c.NP, 2, 4, 128], F8, name="wqkg", tag="wqk")
                nc.sync.dma_start(
                    out=wqkg,
                    in_=wqk_d[g].rearrange(
                        "(pair sub p) (b m) -> p pair sub b m", sub=2, p=128, m=128
                    ),
                )
                wvg = wqkp.tile([128, c.NP, 2, 256], F8, name="wvg", tag="wv")
                nc.sync.dma_start(
                    out=wvg,
                    in_=wv_d[g].rearrange(
                        "(pair sub p) m -> p pair sub m", sub=2, p=128
                    ),
                )
                wvbg = wqkp.tile([128, c.ND, 256], BF16, name="wvbg", tag="wvb")
                nc.sync.dma_start(
                    out=wvbg,
                    in_=wvb_d[g].rearrange("(d p) m -> p d m", p=128),
                )
                # col block b: 0,1 -> Q pairs 2g, 2g+1; 2,3 -> K pairs.
                # K's second kv chunk needs the last LN1 transposes; emit the
                # early V blocks before it so the PE isn't queue-blocked.
                order = [(0, 0), (1, 0), (2, 0), (3, 0), ("v", 0), ("v", 1),
                         ("v", 2), ("v", 3), (2, 512), (3, 512),
                         ("v", 4), ("v", 5), ("v", 6), ("v", 7)]
                def emit_v_block(i):
                    if i < 2:
                        ps = pqk.tile([128, 512], F32, name="ps_v", tag="pqk")
                        for d in range(c.ND):
                            nc.tensor.matmul(
                                ps[:, :256],
                                lhsT=xTb[:, d, i * 128:(i + 1) * 128],
                                rhs=wvbg[:, d, :],
                                start=(d == 0),
                                stop=(d == c.ND - 1),
                            )
                        nc.vector.tensor_scalar_mul(
                            out=Vb[:, i, 4 * g:4 * g + 4, 0:64],
                            in0=ps[:, :256].rearrange("p (h x) -> p h x", x=64),
                            scalar1=kvm_sb[:, i:i + 1],
                        )
                    else:
                        ps = pqk.tile([128, 512], F32, name="ps_v", tag="pqk")
                        for p in range(c.NP):
                            if USE_DR:
                                nc.tensor.matmul(
                                    ps[:, :256],
                                    lhsT=x8T[:, 2 * p:2 * p + 2,
                                             i * 128:(i + 1) * 128],
                                    rhs=wvg[:, p, :, :],
                                    start=(p == 0),
                                    stop=(p == c.NP - 1),
                                    perf_mode=DR,
                                )
                            else:
                                for ii in range(2):
                                    nc.tensor.matmul(
                                        ps[:, :256],
                                        lhsT=x8T[:, 2 * p + ii,
                                                 i * 128:(i + 1) * 128],
                                        rhs=wvg[:, p, ii, :],
                                        start=(p == 0 and ii == 0),
                                        stop=(p == c.NP - 1 and ii == 1),
                                    )
                        nc.vector.tensor_scalar_mul(
                            out=V8[:, i - 2, 4 * g:4 * g + 4, 0:64],
                            in0=ps[:, :256].rearrange("p (h x) -> p h x", x=64),
                            scalar1=kvm_sb[:, i:i + 1],
                        )
                for b, c0 in order:
                    if b == "v":
                        emit_v_block(c0)
                        continue
                    is_q = b < 2
                    hp = 2 * g + (b % 2)
                    w = 512
                    if True:
                        ps = pqk.tile([128, 512], F32, name="ps_qk", tag="pqk")
                        for p in range(c.NP):
                            if USE_DR:
                                nc.tensor.matmul(
                                    ps[:, :w],
                                    lhsT=wqkg[:, p, :, b, :],
                                    rhs=x8T[:, 2 * p:2 * p + 2, c0:c0 + w],
                                    start=(p == 0),
                                    stop=(p == c.NP - 1),
                                    perf_mode=DR,
                                )
                            else:
                                for ii in range(2):
                                    nc.tensor.matmul(
                                        ps[:, :w],
                                        lhsT=wqkg[:, p, ii, b, :],
                                        rhs=x8T[:, 2 * p + ii, c0:c0 + w],
                                        start=(p == 0 and ii == 0),
                                        stop=(p == c.NP - 1 and ii == 1),
                                    )
                        if has_bqk:
                            col = (0 if is_q else c.ND) + 2 * g + (b % 2)
                            nc.vector.tensor_scalar_add(
                                out=ps[:, :w], in0=ps[:, :w],
                                scalar1=bqk_sb[:, col:col + 1],
                            )
                        dst = QT8 if is_q else KT8
                        nc.vector.tensor_copy(
                            out=dst[:, hp, c0:c0 + w], in_=ps[:, :w]
                        )
            def emit_head_attention(h):
                hp, jj = divmod(h, 2)
                p0 = 64 * jj
                # batches in AV accumulation order: prefix pairs full, then
                # diag pair1 (trimmed to q>=256), then diag pair0 (stop).
                batches = [(2, 0, True, False), (3, 0, False, False),
                           (1, 256, False, False), (0, 0, False, True)]
                e8T = e8p.tile([128, c.NKB - 2, c.Sq], F8, name="e8T", tag="e8T")
                ebT = e8p.tile([128, 2, c.Sq], BF16, name="ebT", tag="ebT")
                yps = pyv.tile([128, 512], F32, name="yps", tag="yps")

                def emit_scores(bi):
                    pr, q0, _, _ = batches[bi]
                    psS = pss.tile([128, 2, 512], F32, name="psS", tag="psS")
                    for kk in range(2):
                        kvb = 2 * pr + kk
                        nc.tensor.matmul(
                            psS[:, kk, q0:],
                            lhsT=KT8[p0:p0 + 64, hp, kvb * 128:(kvb + 1) * 128],
                            rhs=QT8[p0:p0 + 64, hp, q0:],
                            start=True,
                            stop=True,
                        )
                    eT = ebT if pr == 0 else e8T
                    co = 0 if pr == 0 else 2 * (pr - 1)
                    nc.scalar.activation(
                        out=eT[:, co:co + 2, q0:],
                        in_=psS[:, :, q0:],
                        func=AF.Exp,
                        scale=SEXP,
                    )
                    if pr < 2:  # diagonal pairs need the causal mask
                        for kk in range(2):
                            kvb = 2 * pr + kk
                            # only cols q0..128*(kvb+1) can be masked
                            qe = 128 * (kvb + 1)
                            nc.gpsimd.affine_select(
                                out=eT[:, co + kk, q0:qe],
                                in_=eT[:, co + kk, q0:qe],
                                pattern=[[1, qe - q0]],
                                compare_op=ALU.is_ge,
                                fill=fill0,
                                base=q0 - 128 * kvb,
                                channel_multiplier=-1,
                            )

                def emit_av(bi):
                    pr, q0, st, sp = batches[bi]
                    if pr == 0:
                        # bf16 pair: kvb1 only sees q>=128 (trimmed, first);
                        # kvb0 full-width last carries the stop flag
                        nc.tensor.matmul(
                            yps[:, 128:],
                            lhsT=Vb[:, 1, h, :],
                            rhs=ebT[:, 1, 128:],
                            start=False,
                            stop=False,
                        )
                        nc.tensor.matmul(
                            yps[:, q0:],
                            lhsT=Vb[:, 0, h, :],
                            rhs=ebT[:, 0, q0:],
                            start=st,
                            stop=sp,
                        )
                    elif USE_DR:
                        co = 2 * (pr - 1)
                        nc.tensor.matmul(
                            yps[:, q0:],
                            lhsT=V8[:, co:co + 2, h, :],
                            rhs=e8T[:, co:co + 2, q0:],
                            start=st,
                            stop=sp,
                            perf_mode=DR,
                        )
                    else:
                        co = 2 * (pr - 1)
                        for kk in range(2):
                            nc.tensor.matmul(
                                yps[:, q0:],
                                lhsT=V8[:, co + kk, h, :],
                                rhs=e8T[:, co + kk, q0:],
                                start=st and kk == 0,
                                stop=sp and kk == 1,
                            )

                # software pipeline: scores run one batch ahead of AV
                emit_scores(0)
                emit_scores(1)
                emit_av(0)
                emit_scores(2)
                emit_av(1)
                emit_scores(3)
                emit_av(2)
                emit_av(3)

                # normalize rows 0..63 by row 64 (broadcast via DRAM bounce)
                rc = ynp.tile([1, c.Sq], F32R, name="rc", tag="rc")
                nc.vector.reciprocal(out=rc, in_=yps[64:65, :])
                rbs = ynp.tile([64, c.Sq], F32R, name="rbs", tag="rbs")
                nc.sync.dma_start(out=dscr[h:h + 1, :], in_=rc)
                _src = dscr[h:h + 1, :]
                nc.gpsimd.dma_start(
                    out=rbs,
                    in_=bass.AP(
                        tensor=_src.tensor,
                        offset=_src.offset,
                        ap=[[0, 64]] + list(_src.ap[1:]),
                    ),
                )
                yTn = ynp.tile([64, c.Sq], F8, name="yTn", tag="yTn")
                yTnb = ynp.tile([64, 128], BF16, name="yTnb", tag="yTnb")
                if has_bv:
                    yTf = ynp.tile([64, c.Sq], F32R, name="yTf", tag="yTf")
                    nc.vector.tensor_mul(out=yTf, in0=yps[0:64, :], in1=rbs)
                    nc.vector.tensor_scalar_add(
                        out=yTf, in0=yTf, scalar1=bv_sb[:, h:h + 1],
                    )
                    nc.vector.tensor_copy(out=yTn, in_=yTf)
                    nc.vector.tensor_copy(out=yTnb, in_=yTf[:, 0:128])
                else:
                    nc.vector.tensor_mul(out=yTn, in0=yps[0:64, :], in1=rbs)
                    nc.vector.tensor_mul(
                        out=yTnb, in0=yps[0:64, 0:128], in1=rbs[:, 0:128]
                    )
                # head h -> yT8 chunk h//2, partitions 64*(h%2)
                nc.sync.dma_start(
                    out=yT8[p0:p0 + 64, hp, :], in_=yTn
                )
                nc.sync.dma_start(
                    out=yT8b[p0:p0 + 64, hp, :], in_=yTnb
                )

            # pipeline: group g GEMMs, then per-head attention; GEMMs of
            # g+1 are emitted between heads to keep the PE fed while ACT exps.
            emit_group_gemms(0)

            # prefetch proj weights during phase B (no deps; keeps the SP
            # queue ahead of the dependent per-head DMAs)
            wp8s, wpbs = [], []
            for c0, w in tiles_of(c.D):
                wp8 = wpp.tile([128, 6, 2, 512], F8, name="wp8", tag="wp")
                nc.sync.dma_start(
                    out=wp8[:, :, :, :w],
                    in_=wproj_d[:, c0:c0 + w].rearrange(
                        "(cc sub p) m -> p cc sub m", sub=2, p=128
                    ),
                )
                wpb = wpbp.tile([128, c.NCC, 512], BF16, name="wpb", tag="wpb")
                nc.sync.dma_start(
                    out=wpb[:, :, :w],
                    in_=wprojb_d[:, c0:c0 + w].rearrange(
                        "(cc p) m -> p cc m", p=128
                    ),
                )
                wp8s.append(wp8)
                wpbs.append(wpb)
            for g in range(c.NG5):
                for j in range(4):
                    if g + 1 < c.NG5 and j == 0:
                        emit_group_gemms(g + 1)
                    emit_head_attention(4 * g + j)

            pyv.release()
            pss.release()
            pqk.release()
            drp.release()
            ynp.release()
            e8p.release()
            qk8p.release()
            wqkp.release()
            x8p.release()

            # -------- Phase C: proj + residual + LN2 + transpose --------
            hzp = tc.alloc_tile_pool(name=f"hzp{rep}", bufs=1)
            h_sb = hzp.tile([128, c.NQB, c.D], F32, name="h_sb")
            zT = hzp.tile([128, c.ND, c.Sq], BF16, name="zT")
            h_true = hzp.tile([128, c.NQB, c.D], F32, name="h_true")

            pb = tc.alloc_tile_pool(name=f"pb{rep}", bufs=6, space="PSUM")
            pt2 = tc.alloc_tile_pool(name=f"pt2{rep}", bufs=2, space="PSUM")

            for ci, (c0, w) in enumerate(tiles_of(c.D)):
                wp8 = wp8s[ci]
                wpb = wpbs[ci]
                # query block 0 (sharp rows on even cores): bf16 path
                psb = pb.tile([128, 512], F32, name="ps_p0", tag="pb")
                for cc in range(c.NCC):
                    nc.tensor.matmul(
                        psb[:, :w],
                        lhsT=yT8b[:, cc, :],
                        rhs=wpb[:, cc, :w],
                        start=(cc == 0),
                        stop=(cc == c.NCC - 1),
                    )
                nc.vector.tensor_add(
                    out=h_sb[:, 0, c0:c0 + w],
                    in0=psb[:, :w],
                    in1=resid32[:, 0, c0:c0 + w],
                )
                for qb in range(1, c.NQB):
                    ps = pb.tile([128, 512], F32, name="ps_p", tag="pb")
                    for cc in range(6):
                        if USE_DR:
                            nc.tensor.matmul(
                                ps[:, :w],
                                lhsT=yT8[:, 2 * cc:2 * cc + 2,
                                         qb * 128:(qb + 1) * 128],
                                rhs=wp8[:, cc, :, :w],
                                start=(cc == 0),
                                stop=(cc == 5),
                                perf_mode=DR,
                            )
                        else:
                            for ii in range(2):
                                nc.tensor.matmul(
                                    ps[:, :w],
                                    lhsT=yT8[:, 2 * cc + ii,
                                             qb * 128:(qb + 1) * 128],
                                    rhs=wp8[:, cc, ii, :w],
                                    start=(cc == 0 and ii == 0),
                                    stop=(cc == 5 and ii == 1),
                                )
                    nc.vector.tensor_add(
                        out=h_sb[:, qb, c0:c0 + w],
                        in0=ps[:, :w],
                        in1=resid32[:, qb, c0:c0 + w],
                    )

            wpp.release()
            wpbp.release()
            ytp.release()
            residp.release()

            for qb in range(c.NQB):
                xhat2 = workp.tile([128, c.D], F32R, name="xhat2", tag="row2")
                layer_norm_tile(h_sb[:, qb, :], xhat2, 100 + qb)
                nc.gpsimd.tensor_scalar_mul(
                    out=h_true[:, qb, :], in0=h_sb[:, qb, :], scalar1=1.0 / SW
                )
                for d0 in range(0, c.ND, 4):
                    nd = min(4, c.ND - d0)
                    pt = pt2.tile([128, 4, 128], F32R, name="pt2", tag="pt2")
                    for k in range(nd):
                        nc.tensor.transpose(
                            (pt[:, k, :]),
                            (xhat2[:, (d0 + k) * 128:(d0 + k + 1) * 128]),
                            (ident_sb),
                        )
                    nc.vector.tensor_copy(
                        out=zT[:, d0:d0 + nd, qb * 128:(qb + 1) * 128],
                        in_=pt[:, 0:nd, :],
                    )

            # -------- Phase D: MLP --------
            gp = tc.alloc_tile_pool(name=f"gp{rep}", bufs=1)
            outp = tc.alloc_tile_pool(name=f"outp{rep}", bufs=1)
            w2p = tc.alloc_tile_pool(name=f"w2p{rep}", bufs=2)
            gT = gp.tile([128, c.NHT, c.Sq], BF16, name="gT")

            fchunks = tiles_of(c.Dff)
            wfs = []
            for g0, gw in fchunks:
                wf = wcache.tile([128, c.ND, 512], BF16, name="wf", tag="wc")
                nc.sync.dma_start(
                    out=wf[:, :, :gw],
                    in_=wfc_d[:, g0:g0 + gw].rearrange(
                        "(d p) m -> p d m", p=128
                    ),
                )
                wfs.append(wf)
            for fi, (g0, gw) in enumerate(fchunks):
                wf = wfs[fi]
                for j in range(gw // 128):
                    ht = g0 // 128 + j
                    ps = pb.tile([128, 512], F32, name="ps_f", tag="pb")
                    for d in range(c.ND):
                        nc.tensor.matmul(
                            ps[:, :c.Sq],
                            lhsT=(wf[:, d, j * 128:(j + 1) * 128]),
                            rhs=(zT[:, d, :]),
                            start=(d == 0),
                            stop=(d == c.ND - 1),
                        )
                    nc.scalar.activation(
                        out=gT[:, ht, :],
                        in_=ps[:, :c.Sq],
                        func=AF.Gelu_apprx_tanh,
                        bias=bfc_sb[:, ht:ht + 1],
                    )

            out_ts = []
            for qs in range(c.NQB):
                ot = outp.tile([128, c.D], F32, name=f"out_{qs}")
                out_ts.append(ot)
            for c0, w in tiles_of(c.D):
                psms = [
                    pb.tile([128, 512], F32, name=f"ps_m_{qs}", tag="pb")
                    for qs in range(c.NQB)
                ]
                wf2s = []
                for htg in range(0, c.NHT, 10):
                    wf2 = w2p.tile([128, 10, 512], BF16, name="wf2", tag="wf2")
                    nc.sync.dma_start(
                        out=wf2[:, :, :w],
                        in_=wfc2_d[htg * 128:(htg + 10) * 128,
                                   c0:c0 + w].rearrange(
                            "(t p) m -> p t m", p=128
                        ),
                    )
                    wf2s.append(wf2)
                for htg in range(0, c.NHT, 10):
                    wf2 = wf2s[htg // 10]
                    for t in range(10):
                        ht = htg + t
                        for qs in range(c.NQB):
                            nc.tensor.matmul(
                                psms[qs][:, :w],
                                lhsT=gT[:, ht, qs * 128:(qs + 1) * 128],
                                rhs=wf2[:, t, :w],
                                start=(ht == 0),
                                stop=(ht == c.NHT - 1),
                            )
                for qs in range(c.NQB):
                    nc.vector.tensor_add(
                        out=out_ts[qs][:, c0:c0 + w],
                        in0=psms[qs][:, :w],
                        in1=h_true[:, qs, c0:c0 + w],
                    )
                    if not has_bfc2:
                        # stream each finished column chunk out immediately
                        nc.sync.dma_start(
                            out=out_d[qs * 128:(qs + 1) * 128, c0:c0 + w],
                            in_=out_ts[qs][:, c0:c0 + w],
                        )
            if has_bfc2:
                for qs in range(c.NQB):
                    nc.vector.tensor_add(
                        out=out_ts[qs], in0=out_ts[qs], in1=bfc2_sb
                    )
                    nc.sync.dma_start(
                        out=out_d[qs * 128:(qs + 1) * 128, :],
                        in_=out_ts[qs],
                    )

            w2p.release()
            outp.release()
            gp.release()
            pt2.release()
            pb.release()
            hzp.release()

        for _rep in range(repeat):
            _emit_phases(_rep)

        wcache.release()
        statp.release()
        workp.release()
        constp.release()

    nc.compile()
    return nc


# ----------------------------------------------------------------------------
# Host-side preparation
# ----------------------------------------------------------------------------

def prep_inputs(cfg: Cfg, hidden_states, attention_mask, ln1_g, ln1_b, w_qkv,
                b_qkv, w_proj, b_proj, ln2_g, ln2_b, w_fc, b_fc, w_fc2, b_fc2):
    """Build per-core in_maps. Returns (in_maps, flags)."""
    import ml_dtypes
    c = cfg
    B = hidden_states.shape[0]
    f32 = np.float32
    f8 = ml_dtypes.float8_e4m3
    bf16 = ml_dtypes.bfloat16

    # fold LN affine params into following matmuls
    wqkv_f = (ln1_g[:, None] * w_qkv).astype(f32)
    bqkv_f = (ln1_b @ w_qkv + b_qkv).astype(f32)
    wfc_f = (ln2_g[:, None] * w_fc).astype(f32)
    bfc_f = (ln2_b @ w_fc + b_fc).astype(f32)

    bq = bqkv_f[0:c.D]
    bk = bqkv_f[c.D:2 * c.D]
    bv = bqkv_f[2 * c.D:3 * c.D]

    has_bqk = bool(np.any(bq) or np.any(bk))
    has_bv = bool(np.any(bv))
    has_bproj = bool(np.any(np.asarray(b_proj)))
    has_bfc2 = bool(np.any(b_fc2))

    # wqk8 [NG5, D, 512]: group g cols = [q heads 4g..4g+3 | k heads 4g..4g+3]
    wqk8 = np.zeros((c.NG5, c.D, 512), f32)
    wv8 = np.zeros((c.NG5, c.D, 256), f32)
    for g in range(c.NG5):
        wqk8[g, :, 0:256] = wqkv_f[:, DH * 4 * g:DH * 4 * (g + 1)]
        wqk8[g, :, 256:512] = wqkv_f[:, c.D + DH * 4 * g:c.D + DH * 4 * (g + 1)]
        wv8[g] = wqkv_f[:, 2 * c.D + 256 * g:2 * c.D + 256 * (g + 1)]

    # proj weights: head-major rows (natural w_proj), padded; optional bproj row
    wproj_p = np.zeros((c.NCC * 128, c.D), f32)
    wproj_p[0:c.D, :] = w_proj
    if has_bproj:
        wproj_p[c.D, :] = b_proj

    shared = {
        "wqk8": (wqk8 * SW).astype(f8),
        "wv8": (wv8 * SW).astype(f8),
        "wvb": (wv8 * SW).astype(bf16),
        "wproj8": (wproj_p * SW).astype(f8),
        "wprojb": (wproj_p * SW).astype(bf16),
        "wfc": wfc_f.astype(bf16),
        "wfc2": np.asarray(w_fc2).astype(bf16),
        "bfc": bfc_f.reshape(c.NHT, 128),
        "ident": np.eye(128, dtype=f32),
    }
    if has_bqk:
        bq8 = np.zeros((c.ND, 128), f32)
        bk8 = np.zeros((c.ND, 128), f32)
        for g in range(c.NG5):
            bq8[2 * g] = bq[DH * 4 * g:DH * 4 * g + 128]
            bq8[2 * g + 1] = bq[DH * 4 * g + 128:DH * 4 * (g + 1)]
            bk8[2 * g] = bk[DH * 4 * g:DH * 4 * g + 128]
            bk8[2 * g + 1] = bk[DH * 4 * g + 128:DH * 4 * (g + 1)]
        shared["bqk"] = np.concatenate([bq8, bk8], axis=0) * SW
    if has_bv:
        shared["bv"] = bv.reshape(c.H, 64)
    if has_bfc2:
        shared["bfc2"] = b_fc2.reshape(1, c.D).astype(f32)

    amask = np.asarray(attention_mask).astype(f32)
    hidden_states = np.asarray(hidden_states)
    in_maps = []
    for core in range(2 * B):
        b, g = divmod(core, 2)
        q0 = g * c.Sq
        hid = np.zeros((c.Skv, c.D), f32)
        kvm = np.zeros((c.Skv,), f32)
        hid[0:c.Sq] = hidden_states[b, q0:q0 + c.Sq]
        kvm[0:c.Sq] = amask[b, q0:q0 + c.Sq]
        if q0 > 0:
            hid[c.Sq:c.Sq + q0] = hidden_states[b, 0:q0]
            kvm[c.Sq:c.Sq + q0] = amask[b, 0:q0]
        m = dict(shared)
        m["hid"] = hid.astype(bf16)
        m["kvmask"] = kvm.reshape(c.NKB, 128)
        in_maps.append(m)
    return in_maps, (has_bqk, has_bv, has_bproj, has_bfc2)


_CACHE = {}
LAST_RESULTS = None


def kernel(**inputs):
    global LAST_RESULTS
    import concourse.bass_utils as bass_utils

    cfg = Cfg()
    inputs = {k: np.asarray(v) for k, v in inputs.items()}
    B = inputs["hidden_states"].shape[0]
    in_maps, flags = prep_inputs(cfg, **inputs)

    key = (cfg, flags)
    if key not in _CACHE:
        _CACHE[key] = build_program(cfg, *flags)
    nc = _CACHE[key]

    res = bass_utils.run_bass_kernel_spmd(
        nc, in_maps, core_ids=list(range(2 * B))
    )
    LAST_RESULTS = res

    out = np.zeros((B, 2 * cfg.Sq, cfg.D), np.float32)
    for core in range(2 * B):
        b, g = divmod(core, 2)
        out[b, g * cfg.Sq:(g + 1) * cfg.Sq] = res.results[core]["out"]
    return out

